# revision 47
# baseline (speedup 1.0000x reference)
"""AttentionalSampler Trainium2 kernel (v2).

Data-parallel over B*T=128 groups: 8 NeuronCores x 16 groups, processed as 8
pairs of groups per core (2 groups stacked on 128 partitions).

Key structure (all 16-bit tensors bf16, accumulation fp32):
  - Every transpose is a coarse DMA XBAR transpose (2-byte dtype): the SBUF
    [p, (c d)] -> [d, c, p] mapping of one dma_start_transpose replaces all
    PE identity-matmul transposes and their PSUM->SBUF copies.
  - Distance bias is precomputed on HOST as exp(bias) and folded into the
    softmax by one fused multiply+row-sum (tensor_tensor_reduce) on DVE.
  - LN rstd = exp(-0.5*ln(var+eps)): Ln and Exp share one ACT table set, so
    the kernel never reloads activation tables after startup.
  - RoPE uses the closed-form 2x2 per-element matrix [T1 T2; T3 T4] of the
    reference's in-place update, applied straight from kproj PSUM (no copy).
  - k LN stats via one square + two tensor_reduce per pair; kz = k*rstd+nmr
    runs on the Scalar engine as Identity activations with per-partition
    scale/bias, freeing DVE.
"""

import numpy as np
import ml_dtypes

D = 128
HP = 32
WP = 32
M = 64
B = 8
T = 16
P = HP * WP
BT = B * T
N_CORES = 8
BT_LOC = BT // N_CORES   # 16 groups per core
NPAIR = BT_LOC // 2      # 8 pairs per core
NCH = P // 128           # 8 chunks of 128 patches per group
DECAY = 2.0
EPS = 1e-5
SQD = float(np.sqrt(np.float32(D)))

F32 = np.float32
BF16 = ml_dtypes.bfloat16

# channel permutation: new j reads old perm[j]; layout [a(32) c(32) | b(32) e(32)]
PERM = np.concatenate([np.arange(0, D, 4), np.arange(1, D, 4),
                       np.arange(2, D, 4), np.arange(3, D, 4)])


def _rope_mats(h, w):
    """T1..T4 [.., 64] for the reference's in-place rope at coords (h, w):
    first-half out = T1*ac + T2*be ; second-half out = T3*ac + T4*be."""
    theta = (100.0 ** (-4.0 * np.arange(1, D // 4 + 1, dtype=np.float64) / D))
    ch = np.cos(theta * h[..., None])
    sh = np.sin(theta * h[..., None])
    cw = np.cos(theta * w[..., None])
    sw = np.sin(theta * w[..., None])
    u = np.concatenate([ch, cw], axis=-1)          # (..., 64)
    v = np.concatenate([sh, -sw], axis=-1)
    return u, -v, -v * u, u + v * v


def _host_k_tables():
    """Static k-side tables. ktab [128, 4, NCH, 64] bf16 (patch = c*128 + p);
    expb_grid [P, P-like] is built per-token later."""
    pidx = np.arange(P)
    h = (pidx // WP).astype(np.float64)
    w = (pidx % WP).astype(np.float64)
    t1, t2, t3, t4 = _rope_mats(h, w)              # (P, 64) each
    ktab = np.stack([t1, t2, t3, t4], axis=1)      # (P, 4, 64)
    ktab = ktab.reshape(NCH, 128, 4, 64).transpose(1, 2, 0, 3)  # (128,4,NCH,64)
    return np.ascontiguousarray(ktab.astype(BF16))


def _host_q_tables(pos_loc):
    """q4 [128, NPAIR, 4, 64] f32 from positions (BT_LOC, M)."""
    ph = (pos_loc // WP).astype(np.float64)
    pw = (pos_loc % WP).astype(np.float64)
    t = np.stack(_rope_mats(ph, pw), axis=2)       # (BT_LOC, M, 4, 64)
    t = t.reshape(NPAIR, 2 * M, 4, 64).transpose(1, 0, 2, 3)
    return np.ascontiguousarray(t.astype(BF16))


def _host_expbias(pos_loc):
    """exp(-dist/(2*DECAY^2)) per token: [NPAIR, 128, P] bf16."""
    ph = (pos_loc // WP).astype(np.float64)        # (BT_LOC, M)
    pw = (pos_loc % WP).astype(np.float64)
    pidx = np.arange(P)
    gh = (pidx // WP).astype(np.float64)
    gw = (pidx % WP).astype(np.float64)
    d2 = (ph[..., None] - gh) ** 2 + (pw[..., None] - gw) ** 2
    eb = np.exp(-np.sqrt(d2) / (2.0 * DECAY ** 2))
    return np.ascontiguousarray(
        eb.reshape(NPAIR, 2 * M, P).transpose(1, 0, 2).astype(BF16))


def _build_program(has_bq, has_bk, has_bln, debug=False, stage=4):
    from contextlib import ExitStack
    import concourse.bass as bass
    import concourse.bacc as bacc
    import concourse.tile as tile
    import concourse.mybir as mybir

    dt = mybir.dt
    ALU = mybir.AluOpType
    ACTF = mybir.ActivationFunctionType

    nc = bacc.Bacc("TRN2", target_bir_lowering=False)

    def din(name, shape, dtype):
        return nc.dram_tensor(name, shape, dtype, kind="ExternalInput").ap()

    t_in = din("t16", [BT_LOC * M, D], dt.bfloat16)
    # partition-major copy: mvp16[p, g, c, d] = mv[g, c*128+p, d];
    # per-partition rows are contiguous -> 128 big descriptors per load
    mvp_in = din("mvp16", [128, BT_LOC, NCH, D], dt.bfloat16)
    # host-pretransposed: mvt16[d, g, c, p] = mv[g, c*128+p, d]
    mvt_in = din("mvt16", [D, BT_LOC, NCH, 128], dt.bfloat16)
    wqt_in = din("wqt", [D, D], dt.bfloat16)
    wkt_in = din("wkt", [D, D], dt.bfloat16)
    ktab_in = din("ktab", [128, 4, NCH, 64], dt.bfloat16)
    q4_in = din("q4", [2 * M, NPAIR, 4, 64], dt.bfloat16)
    expb_in = din("expb", [2 * M, NPAIR, P], dt.bfloat16)
    g2_in = din("g2v", [1, D], dt.float32)
    bg_in = din("bgv", [1, D], dt.float32) if has_bln else None
    gb_in = din("gbv", [1, D], dt.float32) if has_bln else None
    bq_in = din("bqv", [1, D], dt.float32) if has_bq else None
    bk_in = din("bkv", [1, D], dt.float32) if has_bk else None

    out_dram = nc.dram_tensor("out", [BT_LOC, M, D], dt.float32,
                              kind="ExternalOutput").ap()
    if debug:
        dbg = {
            "d_qr": nc.dram_tensor("d_qr", [2 * M, D], dt.float32,
                                   kind="ExternalOutput").ap(),
            "d_qg": nc.dram_tensor("d_qg", [2 * M, D], dt.bfloat16,
                                   kind="ExternalOutput").ap(),
            "d_qgT": nc.dram_tensor("d_qgT", [D, 2 * M], dt.bfloat16,
                                    kind="ExternalOutput").ap(),
            "d_kb": nc.dram_tensor("d_kb", [128, 2 * NCH, D], dt.bfloat16,
                                   kind="ExternalOutput").ap(),
            "d_kz": nc.dram_tensor("d_kz", [128, 2 * NCH, D], dt.bfloat16,
                                   kind="ExternalOutput").ap(),
            "d_kzT": nc.dram_tensor("d_kzT", [128, 2 * NCH, D], dt.bfloat16,
                                    kind="ExternalOutput").ap(),
            "d_aer": nc.dram_tensor("d_aer", [128, P], dt.bfloat16,
                                    kind="ExternalOutput").ap(),
            "d_ae": nc.dram_tensor("d_ae", [128, P], dt.bfloat16,
                                   kind="ExternalOutput").ap(),
            "d_attT": nc.dram_tensor("d_attT", [128, NCH, 128], dt.bfloat16,
                                     kind="ExternalOutput").ap(),
            "d_ssum": nc.dram_tensor("d_ssum", [128, 1], dt.float32,
                                     kind="ExternalOutput").ap(),
            "d_rsk": nc.dram_tensor("d_rsk", [128, 2 * NCH], dt.float32,
                                    kind="ExternalOutput").ap(),
            "d_mvT": nc.dram_tensor("d_mvT", [128, 2 * NCH, D], dt.bfloat16,
                                    kind="ExternalOutput").ap(),
        }

    def bcast(dram_ap, parts=128):
        return bass.AP(tensor=dram_ap.tensor, offset=dram_ap.offset,
                       ap=[[0, parts]] + list(dram_ap.ap[1:]))

    with tile.TileContext(nc) as tc, ExitStack() as ctx:
        singles = ctx.enter_context(tc.tile_pool(name="singles", bufs=1))
        mvp = ctx.enter_context(tc.tile_pool(name="mvp", bufs=3))
        kp = ctx.enter_context(tc.tile_pool(name="kp", bufs=3))
        qp = ctx.enter_context(tc.tile_pool(name="qp", bufs=3))
        ap_ = ctx.enter_context(tc.tile_pool(name="ap", bufs=3))
        smal = ctx.enter_context(tc.tile_pool(name="smal", bufs=4))
        ps_kg = ctx.enter_context(tc.tile_pool(name="ps_kg", bufs=1, space="PSUM"))
        ps_att = ctx.enter_context(tc.tile_pool(name="ps_att", bufs=1, space="PSUM"))
        ps_sm = ctx.enter_context(tc.tile_pool(name="ps_sm", bufs=1, space="PSUM"))

        # ---- resident constants (k-chain deps first, then q deps) ----
        wqt = singles.tile([D, D], dt.bfloat16)
        nc.sync.dma_start(out=wqt, in_=wqt_in)
        # tT_all [d, pair, m128] via one DRAM transpose (2D out AP: row=free)
        tT_all = singles.tile([D, NPAIR, 2 * M], dt.bfloat16)
        nc.sync.dma_start(out=tT_all.rearrange("d i m -> d (i m)"),
                          in_=t_in, transpose=True)
        wkt = singles.tile([D, D], dt.bfloat16)
        nc.sync.dma_start(out=wkt, in_=wkt_in)
        ktab = singles.tile([128, 4, NCH, 64], dt.bfloat16)
        nc.sync.dma_start(out=ktab, in_=ktab_in)
        q4 = singles.tile([2 * M, NPAIR, 4, 64], dt.bfloat16)
        nc.sync.dma_start(out=q4, in_=q4_in)
        g2bc = singles.tile([128, D], dt.float32)
        nc.sync.dma_start(out=g2bc, in_=bcast(g2_in))
        ones_col = singles.tile([128, 1], dt.bfloat16)
        nc.vector.memset(ones_col, 1.0)
        if has_bln:
            bgbc = singles.tile([128, D], dt.float32)
            nc.sync.dma_start(out=bgbc, in_=bcast(bg_in))
            gbbc = singles.tile([128, D], dt.float32)
            nc.sync.dma_start(out=gbbc, in_=bcast(gb_in))
        if has_bq:
            bqbc = singles.tile([128, D], dt.float32)
            nc.sync.dma_start(out=bqbc, in_=bcast(bq_in))
        if has_bk:
            bkbc = singles.tile([128, D], dt.float32)
            nc.sync.dma_start(out=bkbc, in_=bcast(bk_in))
        epst = singles.tile([128, 1], dt.float32)
        nc.vector.memset(epst, EPS)
        # all pairs' exp-bias, resident (16KB/partition); loaded mid-loop so
        # it does not gate the first pairs' mv loads on the DMA chain
        expb = singles.tile([2 * M, NPAIR, P], dt.bfloat16)

        for i in range(NPAIR):
            # ---- loads ----
            mv_b = mvp.tile([128, 2 * NCH, D], dt.bfloat16, tag="mv_b")
            nc.sync.dma_start(
                out=mv_b,
                in_=mvp_in[:, 2 * i:2 * i + 2].rearrange(
                    "p g c d -> p (g c) d"))
            mvT = mvp.tile([D, 2 * NCH, 128], dt.bfloat16, tag="mvT")
            nc.sync.dma_start(
                out=mvT,
                in_=mvt_in[:, 2 * i:2 * i + 2].rearrange(
                    "d g c p -> d (g c) p"))
            if i == 0:
                nc.sync.dma_start(out=expb, in_=expb_in)

            # ---- k path ----
            if stage < 2:
                out_f = qp.tile([128, D], dt.float32, tag="out_f")
                if stage < 1:
                    nc.vector.memset(out_f, 0.0)
                else:
                    nc.vector.tensor_copy(out_f, mv_b[:, 0, 0:D])
                nc.sync.dma_start(
                    out=out_dram[2 * i:2 * i + 2].rearrange(
                        "g m d -> (g m) d"),
                    in_=out_f)
                continue
            k_b = kp.tile([128, 2 * NCH, D], dt.bfloat16, tag="k_b")
            for g in range(2):
                ps_k = ps_kg.tile([128, NCH, D], dt.float32, tag="psk")
                for c in range(NCH):
                    nc.tensor.matmul(ps_k[:, c, :], mvT[:, NCH * g + c, :],
                                     wkt, start=True, stop=True)
                if has_bk:
                    for c in range(NCH):
                        nc.vector.tensor_add(ps_k[:, c, :], ps_k[:, c, :],
                                             bkbc)
                sl = slice(NCH * g, NCH * (g + 1))
                kt1 = kp.tile([128, NCH, 64], dt.bfloat16, tag="kt1")
                kt2 = kp.tile([128, NCH, 64], dt.bfloat16, tag="kt2")
                kt3 = kp.tile([128, NCH, 64], dt.bfloat16, tag="kt3")
                kt4 = kp.tile([128, NCH, 64], dt.bfloat16, tag="kt4")
                nc.vector.tensor_mul(kt1, ktab[:, 0], ps_k[:, :, 0:64])
                nc.vector.tensor_mul(kt2, ktab[:, 1], ps_k[:, :, 64:128])
                nc.vector.tensor_mul(kt3, ktab[:, 2], ps_k[:, :, 0:64])
                nc.vector.tensor_mul(kt4, ktab[:, 3], ps_k[:, :, 64:128])
                nc.gpsimd.tensor_add(k_b[:, sl, 0:64], kt1, kt2)
                nc.gpsimd.tensor_add(k_b[:, sl, 64:128], kt3, kt4)
            # LN stats: mu, var per (p, chunk)
            ksq = kp.tile([128, 2 * NCH, D], dt.bfloat16, tag="ksq")
            nc.scalar.activation(out=ksq, in_=k_b, func=ACTF.Square)
            s1 = smal.tile([128, 2 * NCH], dt.float32, tag="s1")
            nc.vector.tensor_reduce(out=s1, in_=k_b,
                                    axis=mybir.AxisListType.X, op=ALU.add)
            s2 = smal.tile([128, 2 * NCH], dt.float32, tag="s2")
            nc.vector.tensor_reduce(out=s2, in_=ksq,
                                    axis=mybir.AxisListType.X, op=ALU.add)
            mu = smal.tile([128, 2 * NCH], dt.float32, tag="mu")
            nc.vector.tensor_scalar_mul(mu, s1, 1.0 / D)
            musq = smal.tile([128, 2 * NCH], dt.float32, tag="musq")
            nc.vector.tensor_mul(musq, mu, mu)
            var = smal.tile([128, 2 * NCH], dt.float32, tag="var")
            nc.vector.tensor_scalar_mul(var, s2, 1.0 / D)
            nc.vector.tensor_sub(var, var, musq)
            lnk = smal.tile([128, 2 * NCH], dt.float32, tag="lnk")
            nc.scalar.activation(out=lnk, in_=var, func=ACTF.Ln,
                                 bias=epst, scale=1.0)
            rsk = smal.tile([128, 2 * NCH], dt.float32, tag="rsk")
            nc.scalar.activation(out=rsk, in_=lnk, func=ACTF.Exp, scale=-0.5)
            nmr = smal.tile([128, 2 * NCH], dt.float32, tag="nmr")
            nc.vector.tensor_mul(nmr, mu, rsk)
            nc.vector.tensor_scalar_mul(nmr, nmr, -1.0)
            # kz = k*rstd + nmr: 12 chunks on Scalar (Identity with per-chunk
            # scale/bias), 4 on GpSimd, to shorten the serial chain
            kz = kp.tile([128, 2 * NCH, D], dt.bfloat16, tag="kz")
            for c in range(2 * NCH):
                if c % 4 == 3:
                    nc.gpsimd.tensor_scalar(
                        out=kz[:, c, :], in0=k_b[:, c, :],
                        scalar1=rsk[:, c:c + 1], scalar2=nmr[:, c:c + 1],
                        op0=ALU.mult, op1=ALU.add)
                else:
                    nc.scalar.activation(out=kz[:, c, :], in_=k_b[:, c, :],
                                         func=ACTF.Identity,
                                         scale=rsk[:, c:c + 1],
                                         bias=nmr[:, c:c + 1])
            kzT = kp.tile([128, 2 * NCH, D], dt.bfloat16, tag="kzT")
            nc.sync.dma_start(out=kzT,
                              in_=kz.rearrange("p c d -> p (c d)"),
                              transpose=True)

            # ---- q path ----
            ps_q = ps_sm.tile([128, 128], dt.float32, tag="psq")
            nc.tensor.matmul(ps_q, tT_all[:, i, :], wqt, start=True, stop=True)
            if has_bq:
                nc.vector.tensor_add(ps_q, ps_q, bqbc)
            qt1 = qp.tile([2 * M, 64], dt.float32, tag="qt1")
            qt2 = qp.tile([2 * M, 64], dt.float32, tag="qt2")
            qt3 = qp.tile([2 * M, 64], dt.float32, tag="qt3")
            qt4 = qp.tile([2 * M, 64], dt.float32, tag="qt4")
            nc.vector.tensor_mul(qt1, q4[:, i, 0, :], ps_q[:, 0:64])
            nc.vector.tensor_mul(qt2, q4[:, i, 1, :], ps_q[:, 64:128])
            nc.vector.tensor_mul(qt3, q4[:, i, 2, :], ps_q[:, 0:64])
            nc.vector.tensor_mul(qt4, q4[:, i, 3, :], ps_q[:, 64:128])
            q_r = qp.tile([2 * M, D], dt.float32, tag="q_r")
            nc.vector.tensor_add(q_r[:, 0:64], qt1, qt2)
            nc.vector.tensor_add(q_r[:, 64:128], qt3, qt4)
            bnq = smal.tile([2 * M, 6], dt.float32, tag="bnq")
            nc.vector.bn_stats(out=bnq, in_=q_r)
            mvq = smal.tile([2 * M, 2], dt.float32, tag="mvq")
            nc.vector.bn_aggr(out=mvq, in_=bnq)
            lnq = smal.tile([2 * M, 1], dt.float32, tag="lnq")
            nc.scalar.activation(out=lnq, in_=mvq[:, 1:2], func=ACTF.Ln,
                                 bias=epst[0:2 * M], scale=1.0)
            rsq = smal.tile([2 * M, 1], dt.float32, tag="rsq")
            nc.scalar.activation(out=rsq, in_=lnq, func=ACTF.Exp, scale=-0.5)
            qz = qp.tile([2 * M, D], dt.float32, tag="qz")
            nc.vector.tensor_scalar(out=qz, in0=q_r, scalar1=mvq[:, 0:1],
                                    scalar2=rsq, op0=ALU.subtract,
                                    op1=ALU.mult)
            qg = qp.tile([2 * M, D], dt.bfloat16, tag="qg")
            nc.vector.tensor_mul(qg, qz, g2bc)
            if has_bln:
                nc.vector.tensor_add(qg, qg, bgbc)
                cexp = smal.tile([2 * M, 1], dt.float32, tag="cexp")
                trash = qp.tile([2 * M, D], dt.float32, tag="trash")
                nc.vector.tensor_tensor_reduce(
                    out=trash, in0=qz, in1=gbbc, scale=1.0, scalar=0.0,
                    op0=ALU.mult, op1=ALU.add, accum_out=cexp)
            qgT = qp.tile([D, 2 * M], dt.bfloat16, tag="qgT")
            nc.sync.dma_start(out=qgT, in_=qg, transpose=True)

            # ---- attention ----
            if stage < 3:
                out_f = qp.tile([128, D], dt.float32, tag="out_f")
                nc.vector.tensor_copy(out_f, kzT[:, 0, :])
                nc.sync.dma_start(
                    out=out_dram[2 * i:2 * i + 2].rearrange(
                        "g m d -> (g m) d"),
                    in_=out_f)
                continue
            ps_a = ps_att.tile([128, P], dt.float32, tag="att")
            for gi in range(2):
                lhs = qgT[:, gi * M:(gi + 1) * M]
                for h in range(2):
                    rhs = kzT[:, NCH * gi + 4 * h: NCH * gi + 4 * (h + 1), :]
                    nc.tensor.matmul(
                        ps_a[gi * M:(gi + 1) * M, 512 * h:512 * (h + 1)],
                        lhs, rhs, start=True, stop=True)
            if stage == 21:
                out_f = qp.tile([128, D], dt.float32, tag="out_f")
                nc.vector.tensor_copy(out_f, ps_a[:, 0:D])
                nc.sync.dma_start(
                    out=out_dram[2 * i:2 * i + 2].rearrange(
                        "g m d -> (g m) d"),
                    in_=out_f)
                continue
            aer = ap_.tile([128, P], dt.bfloat16, tag="aer")
            if has_bln:
                nc.scalar.activation(out=aer, in_=ps_a, func=ACTF.Exp,
                                     bias=cexp, scale=1.0)
            else:
                nc.scalar.activation(out=aer, in_=ps_a, func=ACTF.Exp,
                                     scale=1.0)
            if stage == 22:
                out_f = qp.tile([128, D], dt.float32, tag="out_f")
                nc.vector.tensor_copy(out_f, aer[:, 0:D])
                nc.sync.dma_start(
                    out=out_dram[2 * i:2 * i + 2].rearrange(
                        "g m d -> (g m) d"),
                    in_=out_f)
                continue
            ae = ap_.tile([128, P], dt.bfloat16, tag="ae")
            nc.gpsimd.tensor_mul(ae, aer, expb[:, i, :])
            if stage == 23:
                out_f = qp.tile([128, D], dt.float32, tag="out_f")
                nc.vector.tensor_copy(out_f, ae[:, 0:D])
                nc.sync.dma_start(
                    out=out_dram[2 * i:2 * i + 2].rearrange(
                        "g m d -> (g m) d"),
                    in_=out_f)
                continue
            attT = ap_.tile([128, NCH, 128], dt.bfloat16, tag="attT")
            nc.sync.dma_start(out=attT, in_=ae, transpose=True)

            # ---- out = attT.T @ mv ----
            if stage < 4:
                out_f = qp.tile([128, D], dt.float32, tag="out_f")
                nc.vector.tensor_copy(out_f, attT[:, 0, :])
                nc.sync.dma_start(
                    out=out_dram[2 * i:2 * i + 2].rearrange(
                        "g m d -> (g m) d"),
                    in_=out_f)
                continue
            # unnormalized out + softmax denom (ones-rhs matmul reuses the
            # already-loaded attT chunk weights; separate PSUM bank)
            ps_o = ps_sm.tile([128, D], dt.float32, tag="out")
            ps_s = ps_sm.tile([128, 1], dt.float32, tag="ssum")
            for gi in range(2):
                for c in range(NCH):
                    lhs = attT[:, c, gi * M:(gi + 1) * M]
                    nc.tensor.matmul(
                        ps_o[gi * M:(gi + 1) * M, :],
                        lhs, mv_b[:, NCH * gi + c, :],
                        start=(c == 0), stop=(c == NCH - 1))
                    nc.tensor.matmul(
                        ps_s[gi * M:(gi + 1) * M, :],
                        lhs, ones_col,
                        start=(c == 0), stop=(c == NCH - 1))
            srec = smal.tile([128, 1], dt.float32, tag="srec")
            nc.vector.reciprocal(srec, ps_s)
            out_f = qp.tile([128, D], dt.float32, tag="out_f")
            nc.vector.tensor_scalar(out=out_f, in0=ps_o,
                                    scalar1=srec, scalar2=None, op0=ALU.mult)
            nc.sync.dma_start(
                out=out_dram[2 * i:2 * i + 2].rearrange("g m d -> (g m) d"),
                in_=out_f)
            if debug and i == 0:
                nc.sync.dma_start(out=dbg["d_qr"], in_=q_r)
                nc.sync.dma_start(out=dbg["d_qg"], in_=qg)
                nc.sync.dma_start(out=dbg["d_qgT"], in_=qgT)
                nc.sync.dma_start(out=dbg["d_kb"], in_=k_b)
                nc.sync.dma_start(out=dbg["d_kz"], in_=kz)
                nc.sync.dma_start(out=dbg["d_kzT"], in_=kzT)
                nc.sync.dma_start(out=dbg["d_aer"], in_=aer)
                nc.sync.dma_start(out=dbg["d_ae"], in_=ae)
                nc.sync.dma_start(out=dbg["d_attT"], in_=attT)
                nc.sync.dma_start(out=dbg["d_ssum"], in_=srec)
                nc.sync.dma_start(out=dbg["d_rsk"], in_=rsk)
                nc.sync.dma_start(out=dbg["d_mvT"], in_=mvT)

    # Pin one activation table set: strip 'exp'/'ln' from every act-func set
    # except natural_log_exp_and_others (list order/length preserved, so the
    # emitted act_func_set_id still indexes the real act_info.json), making
    # the greedy table-load pass choose the shared set for both Ln and Exp —
    # one ACT_TABLE_LOAD for the whole kernel instead of two per pair.
    import concourse.bacc as bacc_mod
    orig_get = bacc_mod.get_activation_tables

    def pinned_get(arch):
        tabs = orig_get(arch)
        out = {}
        for name, funcs in tabs.items():
            if name != "natural_log_exp_and_others":
                funcs = {f for f in funcs
                         if f not in (mybir.ActivationFunctionType.Exp,
                                      mybir.ActivationFunctionType.Ln)}
            out[name] = funcs
        return out

    bacc_mod.get_activation_tables = pinned_get
    try:
        nc.compile()
    finally:
        bacc_mod.get_activation_tables = orig_get
    return nc


_PROG_CACHE = {}

LAST_RESULT = None


def kernel(t, mv, positions, Wq, bq, Wk, bk, ln_g, ln_b, _trace=False):
    global LAST_RESULT
    from concourse.bass_utils import run_bass_kernel_spmd

    t = np.asarray(t, F32).reshape(BT, M, D)
    mv_a = np.ascontiguousarray(np.asarray(mv, F32).reshape(BT, P, D)).astype(BF16)
    pos = np.asarray(positions).reshape(BT, M).astype(np.int64)
    Wq = np.asarray(Wq, F32)
    Wk = np.asarray(Wk, F32)
    bq = np.asarray(bq, F32)
    bk = np.asarray(bk, F32)
    ln_g = np.asarray(ln_g, F32)
    ln_b = np.asarray(ln_b, F32)

    wqt = np.ascontiguousarray(Wq.T[:, PERM]).astype(BF16)
    wkt = np.ascontiguousarray(Wk.T[:, PERM]).astype(BF16)
    g_p = ln_g[PERM]
    b_p = ln_b[PERM]
    bq_p = bq[PERM].astype(F32)
    bk_p = bk[PERM].astype(F32)
    g2v = (g_p * g_p / SQD).astype(F32)[None, :]
    bgv = (b_p * g_p / SQD).astype(F32)[None, :]
    gbv = (g_p * b_p / SQD).astype(F32)[None, :]

    has_bq = bool(np.any(bq_p))
    has_bk = bool(np.any(bk_p))
    has_bln = bool(np.any(b_p))

    key = (has_bq, has_bk, has_bln)
    if key not in _PROG_CACHE:
        _PROG_CACHE[key] = _build_program(*key)
    nc = _PROG_CACHE[key]

    ktab = _host_k_tables()
    t16 = np.ascontiguousarray(t.reshape(N_CORES, BT_LOC * M, D)).astype(BF16)

    in_maps = []
    for ci in range(N_CORES):
        sl = slice(ci * BT_LOC, (ci + 1) * BT_LOC)
        pos_loc = pos[sl]
        mv_chunked = mv_a[sl].reshape(BT_LOC, NCH, 128, D)
        im = {
            "t16": t16[ci],
            "mvp16": np.ascontiguousarray(mv_chunked.transpose(2, 0, 1, 3)),
            "mvt16": np.ascontiguousarray(mv_chunked.transpose(3, 0, 1, 2)),
            "wqt": wqt, "wkt": wkt,
            "ktab": ktab,
            "q4": _host_q_tables(pos_loc),
            "expb": _host_expbias(pos_loc),
            "g2v": g2v,
        }
        if has_bln:
            im["bgv"] = bgv
            im["gbv"] = gbv
        if has_bq:
            im["bqv"] = bq_p[None, :]
        if has_bk:
            im["bkv"] = bk_p[None, :]
        in_maps.append(im)

    res = run_bass_kernel_spmd(nc, in_maps, core_ids=list(range(N_CORES)),
                               trace=_trace)
    LAST_RESULT = res
    out = np.concatenate([r["out"].reshape(BT_LOC, M, D) for r in res.results])
    return out.reshape(B, T, M, D).astype(F32)


# revision 49
# speedup vs baseline: 1.0498x; 1.0498x over previous
"""AttentionalSampler Trainium2 kernel (v2).

Data-parallel over B*T=128 groups: 8 NeuronCores x 16 groups, processed as 8
pairs of groups per core (2 groups stacked on 128 partitions).

Key structure (all 16-bit tensors bf16, accumulation fp32):
  - Every transpose is a coarse DMA XBAR transpose (2-byte dtype): the SBUF
    [p, (c d)] -> [d, c, p] mapping of one dma_start_transpose replaces all
    PE identity-matmul transposes and their PSUM->SBUF copies.
  - Distance bias is precomputed on HOST as exp(bias) and folded into the
    softmax by one fused multiply+row-sum (tensor_tensor_reduce) on DVE.
  - LN rstd = exp(-0.5*ln(var+eps)): Ln and Exp share one ACT table set, so
    the kernel never reloads activation tables after startup.
  - RoPE uses the closed-form 2x2 per-element matrix [T1 T2; T3 T4] of the
    reference's in-place update, applied straight from kproj PSUM (no copy).
  - k LN stats via one square + two tensor_reduce per pair; kz = k*rstd+nmr
    runs on the Scalar engine as Identity activations with per-partition
    scale/bias, freeing DVE.
"""

import numpy as np
import ml_dtypes

D = 128
HP = 32
WP = 32
M = 64
B = 8
T = 16
P = HP * WP
BT = B * T
N_CORES = 8
BT_LOC = BT // N_CORES   # 16 groups per core
NPAIR = BT_LOC // 2      # 8 pairs per core
NCH = P // 128           # 8 chunks of 128 patches per group
DECAY = 2.0
EPS = 1e-5
SQD = float(np.sqrt(np.float32(D)))

F32 = np.float32
BF16 = ml_dtypes.bfloat16

# channel permutation: new j reads old perm[j]; layout [a(32) c(32) | b(32) e(32)]
PERM = np.concatenate([np.arange(0, D, 4), np.arange(1, D, 4),
                       np.arange(2, D, 4), np.arange(3, D, 4)])


def _rope_mats(h, w):
    """T1..T4 [.., 64] for the reference's in-place rope at coords (h, w):
    first-half out = T1*ac + T2*be ; second-half out = T3*ac + T4*be."""
    theta = (100.0 ** (-4.0 * np.arange(1, D // 4 + 1, dtype=np.float64) / D))
    ch = np.cos(theta * h[..., None])
    sh = np.sin(theta * h[..., None])
    cw = np.cos(theta * w[..., None])
    sw = np.sin(theta * w[..., None])
    u = np.concatenate([ch, cw], axis=-1)          # (..., 64)
    v = np.concatenate([sh, -sw], axis=-1)
    return u, -v, -v * u, u + v * v


def _host_k_tables():
    """Static k-side tables. ktab [128, 4, NCH, 64] bf16 (patch = c*128 + p);
    expb_grid [P, P-like] is built per-token later."""
    pidx = np.arange(P)
    h = (pidx // WP).astype(np.float64)
    w = (pidx % WP).astype(np.float64)
    t1, t2, t3, t4 = _rope_mats(h, w)              # (P, 64) each
    ktab = np.stack([t1, t2, t3, t4], axis=1)      # (P, 4, 64)
    ktab = ktab.reshape(NCH, 128, 4, 64).transpose(1, 2, 0, 3)  # (128,4,NCH,64)
    return np.ascontiguousarray(ktab.astype(BF16))


def _host_q_tables(pos_loc):
    """q4 [128, NPAIR, 4, 64] f32 from positions (BT_LOC, M)."""
    ph = (pos_loc // WP).astype(np.float64)
    pw = (pos_loc % WP).astype(np.float64)
    t = np.stack(_rope_mats(ph, pw), axis=2)       # (BT_LOC, M, 4, 64)
    t = t.reshape(NPAIR, 2 * M, 4, 64).transpose(1, 0, 2, 3)
    return np.ascontiguousarray(t.astype(BF16))


def _host_expbias(pos_loc):
    """exp(-dist/(2*DECAY^2)) per token: [NPAIR, 128, P] bf16."""
    ph = (pos_loc // WP).astype(np.float64)        # (BT_LOC, M)
    pw = (pos_loc % WP).astype(np.float64)
    pidx = np.arange(P)
    gh = (pidx // WP).astype(np.float64)
    gw = (pidx % WP).astype(np.float64)
    d2 = (ph[..., None] - gh) ** 2 + (pw[..., None] - gw) ** 2
    eb = np.exp(-np.sqrt(d2) / (2.0 * DECAY ** 2))
    return np.ascontiguousarray(
        eb.reshape(NPAIR, 2 * M, P).transpose(1, 0, 2).astype(BF16))


def _build_program(has_bq, has_bk, has_bln, debug=False, stage=4):
    from contextlib import ExitStack
    import concourse.bass as bass
    import concourse.bacc as bacc
    import concourse.tile as tile
    import concourse.mybir as mybir

    dt = mybir.dt
    ALU = mybir.AluOpType
    ACTF = mybir.ActivationFunctionType

    nc = bacc.Bacc("TRN2", target_bir_lowering=False)

    def din(name, shape, dtype):
        return nc.dram_tensor(name, shape, dtype, kind="ExternalInput").ap()

    t_in = din("t16", [BT_LOC * M, D], dt.bfloat16)
    # partition-major copy: mvp16[p, g, c, d] = mv[g, c*128+p, d];
    # per-partition rows are contiguous -> 128 big descriptors per load
    mvp_in = din("mvp16", [128, BT_LOC, NCH, D], dt.bfloat16)
    # host-pretransposed: mvt16[d, g, c, p] = mv[g, c*128+p, d]
    mvt_in = din("mvt16", [D, BT_LOC, NCH, 128], dt.bfloat16)
    wqt_in = din("wqt", [D, D], dt.bfloat16)
    wkt_in = din("wkt", [D, D], dt.bfloat16)
    ktab_in = din("ktab", [128, 4, NCH, 64], dt.bfloat16)
    q4_in = din("q4", [2 * M, NPAIR, 4, 64], dt.bfloat16)
    expb_in = din("expb", [2 * M, NPAIR, P], dt.bfloat16)
    g2_in = din("g2v", [1, D], dt.float32)
    bg_in = din("bgv", [1, D], dt.float32) if has_bln else None
    gb_in = din("gbv", [1, D], dt.float32) if has_bln else None
    bq_in = din("bqv", [1, D], dt.float32) if has_bq else None
    bk_in = din("bkv", [1, D], dt.float32) if has_bk else None

    out_dram = nc.dram_tensor("out", [BT_LOC, M, D], dt.float32,
                              kind="ExternalOutput").ap()
    if debug:
        dbg = {
            "d_qr": nc.dram_tensor("d_qr", [2 * M, D], dt.float32,
                                   kind="ExternalOutput").ap(),
            "d_qg": nc.dram_tensor("d_qg", [2 * M, D], dt.bfloat16,
                                   kind="ExternalOutput").ap(),
            "d_qgT": nc.dram_tensor("d_qgT", [D, 2 * M], dt.bfloat16,
                                    kind="ExternalOutput").ap(),
            "d_kb": nc.dram_tensor("d_kb", [128, 2 * NCH, D], dt.bfloat16,
                                   kind="ExternalOutput").ap(),
            "d_kz": nc.dram_tensor("d_kz", [128, 2 * NCH, D], dt.bfloat16,
                                   kind="ExternalOutput").ap(),
            "d_kzT": nc.dram_tensor("d_kzT", [128, 2 * NCH, D], dt.bfloat16,
                                    kind="ExternalOutput").ap(),
            "d_aer": nc.dram_tensor("d_aer", [128, P], dt.bfloat16,
                                    kind="ExternalOutput").ap(),
            "d_ae": nc.dram_tensor("d_ae", [128, P], dt.bfloat16,
                                   kind="ExternalOutput").ap(),
            "d_attT": nc.dram_tensor("d_attT", [128, NCH, 128], dt.bfloat16,
                                     kind="ExternalOutput").ap(),
            "d_ssum": nc.dram_tensor("d_ssum", [128, 1], dt.float32,
                                     kind="ExternalOutput").ap(),
            "d_rsk": nc.dram_tensor("d_rsk", [128, 2 * NCH], dt.float32,
                                    kind="ExternalOutput").ap(),
            "d_mvT": nc.dram_tensor("d_mvT", [128, 2 * NCH, D], dt.bfloat16,
                                    kind="ExternalOutput").ap(),
        }

    def bcast(dram_ap, parts=128):
        return bass.AP(tensor=dram_ap.tensor, offset=dram_ap.offset,
                       ap=[[0, parts]] + list(dram_ap.ap[1:]))

    with tile.TileContext(nc) as tc, ExitStack() as ctx:
        singles = ctx.enter_context(tc.tile_pool(name="singles", bufs=1))
        mvp = ctx.enter_context(tc.tile_pool(name="mvp", bufs=3))
        kp = ctx.enter_context(tc.tile_pool(name="kp", bufs=3))
        qp = ctx.enter_context(tc.tile_pool(name="qp", bufs=3))
        ap_ = ctx.enter_context(tc.tile_pool(name="ap", bufs=3))
        smal = ctx.enter_context(tc.tile_pool(name="smal", bufs=4))
        ps_kg = ctx.enter_context(tc.tile_pool(name="ps_kg", bufs=2, space="PSUM"))
        ps_att = ctx.enter_context(tc.tile_pool(name="ps_att", bufs=1, space="PSUM"))
        ps_sm = ctx.enter_context(tc.tile_pool(name="ps_sm", bufs=1, space="PSUM"))

        # ---- resident constants (k-chain deps first, then q deps) ----
        wqt = singles.tile([D, D], dt.bfloat16)
        nc.sync.dma_start(out=wqt, in_=wqt_in)
        # tT_all [d, pair, m128] via one DRAM transpose (2D out AP: row=free)
        tT_all = singles.tile([D, NPAIR, 2 * M], dt.bfloat16)
        nc.sync.dma_start(out=tT_all.rearrange("d i m -> d (i m)"),
                          in_=t_in, transpose=True)
        wkt = singles.tile([D, D], dt.bfloat16)
        nc.sync.dma_start(out=wkt, in_=wkt_in)
        ktab = singles.tile([128, 4, NCH, 64], dt.bfloat16)
        nc.sync.dma_start(out=ktab, in_=ktab_in)
        q4 = singles.tile([2 * M, NPAIR, 4, 64], dt.bfloat16)
        nc.sync.dma_start(out=q4, in_=q4_in)
        g2bc = singles.tile([128, D], dt.float32)
        nc.sync.dma_start(out=g2bc, in_=bcast(g2_in))
        ones_col = singles.tile([128, 1], dt.bfloat16)
        nc.vector.memset(ones_col, 1.0)
        if has_bln:
            bgbc = singles.tile([128, D], dt.float32)
            nc.sync.dma_start(out=bgbc, in_=bcast(bg_in))
            gbbc = singles.tile([128, D], dt.float32)
            nc.sync.dma_start(out=gbbc, in_=bcast(gb_in))
        if has_bq:
            bqbc = singles.tile([128, D], dt.float32)
            nc.sync.dma_start(out=bqbc, in_=bcast(bq_in))
        if has_bk:
            bkbc = singles.tile([128, D], dt.float32)
            nc.sync.dma_start(out=bkbc, in_=bcast(bk_in))
        epst = singles.tile([128, 1], dt.float32)
        nc.vector.memset(epst, EPS)
        # all pairs' exp-bias, resident (16KB/partition); loaded mid-loop so
        # it does not gate the first pairs' mv loads on the DMA chain
        expb = singles.tile([2 * M, NPAIR, P], dt.bfloat16)

        for i in range(NPAIR):
            # ---- loads ----
            mv_b = mvp.tile([128, 2 * NCH, D], dt.bfloat16, tag="mv_b")
            nc.sync.dma_start(
                out=mv_b,
                in_=mvp_in[:, 2 * i:2 * i + 2].rearrange(
                    "p g c d -> p (g c) d"))
            mvT = mvp.tile([D, 2 * NCH, 128], dt.bfloat16, tag="mvT")
            nc.sync.dma_start(
                out=mvT,
                in_=mvt_in[:, 2 * i:2 * i + 2].rearrange(
                    "d g c p -> d (g c) p"))
            if i == 0:
                nc.sync.dma_start(out=expb, in_=expb_in)

            # ---- k path ----
            if stage < 2:
                out_f = qp.tile([128, D], dt.float32, tag="out_f")
                if stage < 1:
                    nc.vector.memset(out_f, 0.0)
                else:
                    nc.vector.tensor_copy(out_f, mv_b[:, 0, 0:D])
                nc.sync.dma_start(
                    out=out_dram[2 * i:2 * i + 2].rearrange(
                        "g m d -> (g m) d"),
                    in_=out_f)
                continue
            k_b = kp.tile([128, 2 * NCH, D], dt.bfloat16, tag="k_b")
            for g in range(2):
                ps_k = ps_kg.tile([128, NCH, D], dt.float32, tag="psk")
                for c in range(NCH):
                    nc.tensor.matmul(ps_k[:, c, :], mvT[:, NCH * g + c, :],
                                     wkt, start=True, stop=True)
                if has_bk:
                    for c in range(NCH):
                        nc.vector.tensor_add(ps_k[:, c, :], ps_k[:, c, :],
                                             bkbc)
                sl = slice(NCH * g, NCH * (g + 1))
                kt1 = kp.tile([128, NCH, 64], dt.bfloat16, tag="kt1")
                kt2 = kp.tile([128, NCH, 64], dt.bfloat16, tag="kt2")
                kt3 = kp.tile([128, NCH, 64], dt.bfloat16, tag="kt3")
                kt4 = kp.tile([128, NCH, 64], dt.bfloat16, tag="kt4")
                nc.vector.tensor_mul(kt1, ktab[:, 0], ps_k[:, :, 0:64])
                nc.vector.tensor_mul(kt2, ktab[:, 1], ps_k[:, :, 64:128])
                nc.vector.tensor_mul(kt3, ktab[:, 2], ps_k[:, :, 0:64])
                nc.vector.tensor_mul(kt4, ktab[:, 3], ps_k[:, :, 64:128])
                nc.gpsimd.tensor_add(k_b[:, sl, 0:64], kt1, kt2)
                nc.gpsimd.tensor_add(k_b[:, sl, 64:128], kt3, kt4)
            # LN stats: mu, var per (p, chunk)
            ksq = kp.tile([128, 2 * NCH, D], dt.bfloat16, tag="ksq")
            nc.scalar.activation(out=ksq, in_=k_b, func=ACTF.Square)
            s1 = smal.tile([128, 2 * NCH], dt.float32, tag="s1")
            nc.vector.tensor_reduce(out=s1, in_=k_b,
                                    axis=mybir.AxisListType.X, op=ALU.add)
            s2 = smal.tile([128, 2 * NCH], dt.float32, tag="s2")
            nc.vector.tensor_reduce(out=s2, in_=ksq,
                                    axis=mybir.AxisListType.X, op=ALU.add)
            mu = smal.tile([128, 2 * NCH], dt.float32, tag="mu")
            nc.vector.tensor_scalar_mul(mu, s1, 1.0 / D)
            musq = smal.tile([128, 2 * NCH], dt.float32, tag="musq")
            nc.vector.tensor_mul(musq, mu, mu)
            var = smal.tile([128, 2 * NCH], dt.float32, tag="var")
            nc.vector.tensor_scalar_mul(var, s2, 1.0 / D)
            nc.vector.tensor_sub(var, var, musq)
            lnk = smal.tile([128, 2 * NCH], dt.float32, tag="lnk")
            nc.scalar.activation(out=lnk, in_=var, func=ACTF.Ln,
                                 bias=epst, scale=1.0)
            rsk = smal.tile([128, 2 * NCH], dt.float32, tag="rsk")
            nc.scalar.activation(out=rsk, in_=lnk, func=ACTF.Exp, scale=-0.5)
            nmr = smal.tile([128, 2 * NCH], dt.float32, tag="nmr")
            nc.vector.tensor_mul(nmr, mu, rsk)
            nc.vector.tensor_scalar_mul(nmr, nmr, -1.0)
            # kz = k*rstd + nmr: 12 chunks on Scalar (Identity with per-chunk
            # scale/bias), 4 on GpSimd, to shorten the serial chain
            kz = kp.tile([128, 2 * NCH, D], dt.bfloat16, tag="kz")
            for c in range(2 * NCH):
                if c % 4 == 3:
                    nc.gpsimd.tensor_scalar(
                        out=kz[:, c, :], in0=k_b[:, c, :],
                        scalar1=rsk[:, c:c + 1], scalar2=nmr[:, c:c + 1],
                        op0=ALU.mult, op1=ALU.add)
                else:
                    nc.scalar.activation(out=kz[:, c, :], in_=k_b[:, c, :],
                                         func=ACTF.Identity,
                                         scale=rsk[:, c:c + 1],
                                         bias=nmr[:, c:c + 1])
            kzT = kp.tile([128, 2 * NCH, D], dt.bfloat16, tag="kzT")
            nc.sync.dma_start(out=kzT,
                              in_=kz.rearrange("p c d -> p (c d)"),
                              transpose=True)

            # ---- q path (qproj borrows the att PSUM tile; lifetimes are
            # disjoint within a pair and ps_att bufs=1 already orders pairs) --
            ps_qa = ps_att.tile([128, P], dt.float32, tag="att")
            ps_q = ps_qa[:, 0:128]
            nc.tensor.matmul(ps_q, tT_all[:, i, :], wqt, start=True, stop=True)
            if has_bq:
                nc.vector.tensor_add(ps_q, ps_q, bqbc)
            qt1 = qp.tile([2 * M, 64], dt.float32, tag="qt1")
            qt2 = qp.tile([2 * M, 64], dt.float32, tag="qt2")
            qt3 = qp.tile([2 * M, 64], dt.float32, tag="qt3")
            qt4 = qp.tile([2 * M, 64], dt.float32, tag="qt4")
            nc.vector.tensor_mul(qt1, q4[:, i, 0, :], ps_q[:, 0:64])
            nc.vector.tensor_mul(qt2, q4[:, i, 1, :], ps_q[:, 64:128])
            nc.vector.tensor_mul(qt3, q4[:, i, 2, :], ps_q[:, 0:64])
            nc.vector.tensor_mul(qt4, q4[:, i, 3, :], ps_q[:, 64:128])
            q_r = qp.tile([2 * M, D], dt.float32, tag="q_r")
            nc.vector.tensor_add(q_r[:, 0:64], qt1, qt2)
            nc.vector.tensor_add(q_r[:, 64:128], qt3, qt4)
            bnq = smal.tile([2 * M, 6], dt.float32, tag="bnq")
            nc.vector.bn_stats(out=bnq, in_=q_r)
            mvq = smal.tile([2 * M, 2], dt.float32, tag="mvq")
            nc.vector.bn_aggr(out=mvq, in_=bnq)
            lnq = smal.tile([2 * M, 1], dt.float32, tag="lnq")
            nc.scalar.activation(out=lnq, in_=mvq[:, 1:2], func=ACTF.Ln,
                                 bias=epst[0:2 * M], scale=1.0)
            rsq = smal.tile([2 * M, 1], dt.float32, tag="rsq")
            nc.scalar.activation(out=rsq, in_=lnq, func=ACTF.Exp, scale=-0.5)
            qz = qp.tile([2 * M, D], dt.float32, tag="qz")
            nc.vector.tensor_scalar(out=qz, in0=q_r, scalar1=mvq[:, 0:1],
                                    scalar2=rsq, op0=ALU.subtract,
                                    op1=ALU.mult)
            qg = qp.tile([2 * M, D], dt.bfloat16, tag="qg")
            nc.vector.tensor_mul(qg, qz, g2bc)
            if has_bln:
                nc.vector.tensor_add(qg, qg, bgbc)
                cexp = smal.tile([2 * M, 1], dt.float32, tag="cexp")
                trash = qp.tile([2 * M, D], dt.float32, tag="trash")
                nc.vector.tensor_tensor_reduce(
                    out=trash, in0=qz, in1=gbbc, scale=1.0, scalar=0.0,
                    op0=ALU.mult, op1=ALU.add, accum_out=cexp)
            qgT = qp.tile([D, 2 * M], dt.bfloat16, tag="qgT")
            nc.sync.dma_start(out=qgT, in_=qg, transpose=True)

            # ---- attention ----
            if stage < 3:
                out_f = qp.tile([128, D], dt.float32, tag="out_f")
                nc.vector.tensor_copy(out_f, kzT[:, 0, :])
                nc.sync.dma_start(
                    out=out_dram[2 * i:2 * i + 2].rearrange(
                        "g m d -> (g m) d"),
                    in_=out_f)
                continue
            ps_a = ps_att.tile([128, P], dt.float32, tag="att")
            for gi in range(2):
                lhs = qgT[:, gi * M:(gi + 1) * M]
                for h in range(2):
                    rhs = kzT[:, NCH * gi + 4 * h: NCH * gi + 4 * (h + 1), :]
                    nc.tensor.matmul(
                        ps_a[gi * M:(gi + 1) * M, 512 * h:512 * (h + 1)],
                        lhs, rhs, start=True, stop=True)
            if stage == 21:
                out_f = qp.tile([128, D], dt.float32, tag="out_f")
                nc.vector.tensor_copy(out_f, ps_a[:, 0:D])
                nc.sync.dma_start(
                    out=out_dram[2 * i:2 * i + 2].rearrange(
                        "g m d -> (g m) d"),
                    in_=out_f)
                continue
            aer = ap_.tile([128, P], dt.bfloat16, tag="aer")
            if has_bln:
                nc.scalar.activation(out=aer, in_=ps_a, func=ACTF.Exp,
                                     bias=cexp, scale=1.0)
            else:
                nc.scalar.activation(out=aer, in_=ps_a, func=ACTF.Exp,
                                     scale=1.0)
            if stage == 22:
                out_f = qp.tile([128, D], dt.float32, tag="out_f")
                nc.vector.tensor_copy(out_f, aer[:, 0:D])
                nc.sync.dma_start(
                    out=out_dram[2 * i:2 * i + 2].rearrange(
                        "g m d -> (g m) d"),
                    in_=out_f)
                continue
            ae = ap_.tile([128, P], dt.bfloat16, tag="ae")
            nc.gpsimd.tensor_mul(ae, aer, expb[:, i, :])
            if stage == 23:
                out_f = qp.tile([128, D], dt.float32, tag="out_f")
                nc.vector.tensor_copy(out_f, ae[:, 0:D])
                nc.sync.dma_start(
                    out=out_dram[2 * i:2 * i + 2].rearrange(
                        "g m d -> (g m) d"),
                    in_=out_f)
                continue
            attT = ap_.tile([128, NCH, 128], dt.bfloat16, tag="attT")
            nc.sync.dma_start(out=attT, in_=ae, transpose=True)

            # ---- out = attT.T @ mv ----
            if stage < 4:
                out_f = qp.tile([128, D], dt.float32, tag="out_f")
                nc.vector.tensor_copy(out_f, attT[:, 0, :])
                nc.sync.dma_start(
                    out=out_dram[2 * i:2 * i + 2].rearrange(
                        "g m d -> (g m) d"),
                    in_=out_f)
                continue
            # unnormalized out + softmax denom (ones-rhs matmul reuses the
            # already-loaded attT chunk weights; separate PSUM bank)
            ps_o = ps_sm.tile([128, D], dt.float32, tag="out")
            ps_s = ps_sm.tile([128, 1], dt.float32, tag="ssum")
            for gi in range(2):
                for c in range(NCH):
                    lhs = attT[:, c, gi * M:(gi + 1) * M]
                    nc.tensor.matmul(
                        ps_o[gi * M:(gi + 1) * M, :],
                        lhs, mv_b[:, NCH * gi + c, :],
                        start=(c == 0), stop=(c == NCH - 1))
                    nc.tensor.matmul(
                        ps_s[gi * M:(gi + 1) * M, :],
                        lhs, ones_col,
                        start=(c == 0), stop=(c == NCH - 1))
            srec = smal.tile([128, 1], dt.float32, tag="srec")
            nc.vector.reciprocal(srec, ps_s)
            out_f = qp.tile([128, D], dt.float32, tag="out_f")
            nc.vector.tensor_scalar(out=out_f, in0=ps_o,
                                    scalar1=srec, scalar2=None, op0=ALU.mult)
            nc.sync.dma_start(
                out=out_dram[2 * i:2 * i + 2].rearrange("g m d -> (g m) d"),
                in_=out_f)
            if debug and i == 0:
                nc.sync.dma_start(out=dbg["d_qr"], in_=q_r)
                nc.sync.dma_start(out=dbg["d_qg"], in_=qg)
                nc.sync.dma_start(out=dbg["d_qgT"], in_=qgT)
                nc.sync.dma_start(out=dbg["d_kb"], in_=k_b)
                nc.sync.dma_start(out=dbg["d_kz"], in_=kz)
                nc.sync.dma_start(out=dbg["d_kzT"], in_=kzT)
                nc.sync.dma_start(out=dbg["d_aer"], in_=aer)
                nc.sync.dma_start(out=dbg["d_ae"], in_=ae)
                nc.sync.dma_start(out=dbg["d_attT"], in_=attT)
                nc.sync.dma_start(out=dbg["d_ssum"], in_=srec)
                nc.sync.dma_start(out=dbg["d_rsk"], in_=rsk)
                nc.sync.dma_start(out=dbg["d_mvT"], in_=mvT)

    # Pin one activation table set: strip 'exp'/'ln' from every act-func set
    # except natural_log_exp_and_others (list order/length preserved, so the
    # emitted act_func_set_id still indexes the real act_info.json), making
    # the greedy table-load pass choose the shared set for both Ln and Exp —
    # one ACT_TABLE_LOAD for the whole kernel instead of two per pair.
    import concourse.bacc as bacc_mod
    orig_get = bacc_mod.get_activation_tables

    def pinned_get(arch):
        tabs = orig_get(arch)
        out = {}
        for name, funcs in tabs.items():
            if name != "natural_log_exp_and_others":
                funcs = {f for f in funcs
                         if f not in (mybir.ActivationFunctionType.Exp,
                                      mybir.ActivationFunctionType.Ln)}
            out[name] = funcs
        return out

    bacc_mod.get_activation_tables = pinned_get
    try:
        nc.compile()
    finally:
        bacc_mod.get_activation_tables = orig_get
    return nc


_PROG_CACHE = {}

LAST_RESULT = None


def kernel(t, mv, positions, Wq, bq, Wk, bk, ln_g, ln_b, _trace=False):
    global LAST_RESULT
    from concourse.bass_utils import run_bass_kernel_spmd

    t = np.asarray(t, F32).reshape(BT, M, D)
    mv_a = np.ascontiguousarray(np.asarray(mv, F32).reshape(BT, P, D)).astype(BF16)
    pos = np.asarray(positions).reshape(BT, M).astype(np.int64)
    Wq = np.asarray(Wq, F32)
    Wk = np.asarray(Wk, F32)
    bq = np.asarray(bq, F32)
    bk = np.asarray(bk, F32)
    ln_g = np.asarray(ln_g, F32)
    ln_b = np.asarray(ln_b, F32)

    wqt = np.ascontiguousarray(Wq.T[:, PERM]).astype(BF16)
    wkt = np.ascontiguousarray(Wk.T[:, PERM]).astype(BF16)
    g_p = ln_g[PERM]
    b_p = ln_b[PERM]
    bq_p = bq[PERM].astype(F32)
    bk_p = bk[PERM].astype(F32)
    g2v = (g_p * g_p / SQD).astype(F32)[None, :]
    bgv = (b_p * g_p / SQD).astype(F32)[None, :]
    gbv = (g_p * b_p / SQD).astype(F32)[None, :]

    has_bq = bool(np.any(bq_p))
    has_bk = bool(np.any(bk_p))
    has_bln = bool(np.any(b_p))

    key = (has_bq, has_bk, has_bln)
    if key not in _PROG_CACHE:
        _PROG_CACHE[key] = _build_program(*key)
    nc = _PROG_CACHE[key]

    ktab = _host_k_tables()
    t16 = np.ascontiguousarray(t.reshape(N_CORES, BT_LOC * M, D)).astype(BF16)

    in_maps = []
    for ci in range(N_CORES):
        sl = slice(ci * BT_LOC, (ci + 1) * BT_LOC)
        pos_loc = pos[sl]
        mv_chunked = mv_a[sl].reshape(BT_LOC, NCH, 128, D)
        im = {
            "t16": t16[ci],
            "mvp16": np.ascontiguousarray(mv_chunked.transpose(2, 0, 1, 3)),
            "mvt16": np.ascontiguousarray(mv_chunked.transpose(3, 0, 1, 2)),
            "wqt": wqt, "wkt": wkt,
            "ktab": ktab,
            "q4": _host_q_tables(pos_loc),
            "expb": _host_expbias(pos_loc),
            "g2v": g2v,
        }
        if has_bln:
            im["bgv"] = bgv
            im["gbv"] = gbv
        if has_bq:
            im["bqv"] = bq_p[None, :]
        if has_bk:
            im["bkv"] = bk_p[None, :]
        in_maps.append(im)

    res = run_bass_kernel_spmd(nc, in_maps, core_ids=list(range(N_CORES)),
                               trace=_trace)
    LAST_RESULT = res
    out = np.concatenate([r["out"].reshape(BT_LOC, M, D) for r in res.results])
    return out.reshape(B, T, M, D).astype(F32)


# revision 55
# speedup vs baseline: 1.1291x; 1.0755x over previous
"""AttentionalSampler Trainium2 kernel (v2).

Data-parallel over B*T=128 groups: 8 NeuronCores x 16 groups, processed as 8
pairs of groups per core (2 groups stacked on 128 partitions).

Key structure (all 16-bit tensors bf16, accumulation fp32):
  - Every transpose is a coarse DMA XBAR transpose (2-byte dtype): the SBUF
    [p, (c d)] -> [d, c, p] mapping of one dma_start_transpose replaces all
    PE identity-matmul transposes and their PSUM->SBUF copies.
  - Distance bias is precomputed on HOST as exp(bias) and folded into the
    softmax by one fused multiply+row-sum (tensor_tensor_reduce) on DVE.
  - LN rstd = exp(-0.5*ln(var+eps)): Ln and Exp share one ACT table set, so
    the kernel never reloads activation tables after startup.
  - RoPE uses the closed-form 2x2 per-element matrix [T1 T2; T3 T4] of the
    reference's in-place update, applied straight from kproj PSUM (no copy).
  - k LN stats via one square + two tensor_reduce per pair; kz = k*rstd+nmr
    runs on the Scalar engine as Identity activations with per-partition
    scale/bias, freeing DVE.
"""

import numpy as np
import ml_dtypes

D = 128
HP = 32
WP = 32
M = 64
B = 8
T = 16
P = HP * WP
BT = B * T
N_CORES = 8
BT_LOC = BT // N_CORES   # 16 groups per core
NPAIR = BT_LOC // 2      # 8 pairs per core
NCH = P // 128           # 8 chunks of 128 patches per group
DECAY = 2.0
EPS = 1e-5
SQD = float(np.sqrt(np.float32(D)))

F32 = np.float32
BF16 = ml_dtypes.bfloat16

# channel permutation: new j reads old perm[j]; layout [a(32) c(32) | b(32) e(32)]
PERM = np.concatenate([np.arange(0, D, 4), np.arange(1, D, 4),
                       np.arange(2, D, 4), np.arange(3, D, 4)])


def _rope_mats(h, w):
    """T1..T4 [.., 64] for the reference's in-place rope at coords (h, w):
    first-half out = T1*ac + T2*be ; second-half out = T3*ac + T4*be."""
    theta = (100.0 ** (-4.0 * np.arange(1, D // 4 + 1, dtype=np.float64) / D))
    ch = np.cos(theta * h[..., None])
    sh = np.sin(theta * h[..., None])
    cw = np.cos(theta * w[..., None])
    sw = np.sin(theta * w[..., None])
    u = np.concatenate([ch, cw], axis=-1)          # (..., 64)
    v = np.concatenate([sh, -sw], axis=-1)
    return u, -v, -v * u, u + v * v


def _host_k_tables():
    """Static k-side tables. ktab [128, 4, NCH, 64] bf16 (patch = c*128 + p);
    expb_grid [P, P-like] is built per-token later."""
    pidx = np.arange(P)
    h = (pidx // WP).astype(np.float64)
    w = (pidx % WP).astype(np.float64)
    t1, t2, t3, t4 = _rope_mats(h, w)              # (P, 64) each
    ktab = np.stack([t1, t2, t3, t4], axis=1)      # (P, 4, 64)
    ktab = ktab.reshape(NCH, 128, 4, 64).transpose(1, 2, 0, 3)  # (128,4,NCH,64)
    return np.ascontiguousarray(ktab.astype(BF16))


def _host_q_tables(pos_loc):
    """q4 [128, NPAIR, 4, 64] f32 from positions (BT_LOC, M)."""
    ph = (pos_loc // WP).astype(np.float64)
    pw = (pos_loc % WP).astype(np.float64)
    t = np.stack(_rope_mats(ph, pw), axis=2)       # (BT_LOC, M, 4, 64)
    t = t.reshape(NPAIR, 2 * M, 4, 64).transpose(1, 0, 2, 3)
    return np.ascontiguousarray(t.astype(BF16))


def _host_expbias(pos_loc):
    """exp(-dist/(2*DECAY^2)) per token: [NPAIR, 128, P] bf16."""
    ph = (pos_loc // WP).astype(np.float64)        # (BT_LOC, M)
    pw = (pos_loc % WP).astype(np.float64)
    pidx = np.arange(P)
    gh = (pidx // WP).astype(np.float64)
    gw = (pidx % WP).astype(np.float64)
    d2 = (ph[..., None] - gh) ** 2 + (pw[..., None] - gw) ** 2
    eb = np.exp(-np.sqrt(d2) / (2.0 * DECAY ** 2))
    return np.ascontiguousarray(
        eb.reshape(NPAIR, 2 * M, P).transpose(1, 0, 2).astype(BF16))


def _build_program(has_bq, has_bk, has_bln, debug=False, stage=4):
    from contextlib import ExitStack
    import concourse.bass as bass
    import concourse.bacc as bacc
    import concourse.tile as tile
    import concourse.mybir as mybir

    dt = mybir.dt
    ALU = mybir.AluOpType
    ACTF = mybir.ActivationFunctionType

    nc = bacc.Bacc("TRN2", target_bir_lowering=False)

    def din(name, shape, dtype):
        return nc.dram_tensor(name, shape, dtype, kind="ExternalInput").ap()

    t_in = din("t16", [BT_LOC * M, D], dt.bfloat16)
    # partition-major copy: mvp16[p, g, c, 0:128] = mv[g, c*128+p, :] with
    # col 128 = 1.0 (softmax-denominator column for the out matmul); rows
    # are contiguous per partition -> 128 big descriptors per load
    mvp_in = din("mvp16", [128, BT_LOC, NCH, D + 4], dt.bfloat16)
    # host-pretransposed: mvt16[d, g, c, p] = mv[g, c*128+p, d]
    mvt_in = din("mvt16", [D, BT_LOC, NCH, 128], dt.bfloat16)
    wqt_in = din("wqt", [D, D], dt.bfloat16)
    wkt_in = din("wkt", [D, D], dt.bfloat16)
    ktab_in = din("ktab", [128, 4, NCH, 64], dt.bfloat16)
    q4_in = din("q4", [2 * M, NPAIR, 4, 64], dt.bfloat16)
    expb_in = din("expb", [2 * M, NPAIR, P], dt.bfloat16)
    g2_in = din("g2v", [1, D], dt.float32)
    bg_in = din("bgv", [1, D], dt.float32) if has_bln else None
    gb_in = din("gbv", [1, D], dt.float32) if has_bln else None
    bq_in = din("bqv", [1, D], dt.float32) if has_bq else None
    bk_in = din("bkv", [1, D], dt.float32) if has_bk else None

    out_dram = nc.dram_tensor("out", [BT_LOC, M, D], dt.float32,
                              kind="ExternalOutput").ap()
    if debug:
        dbg = {
            "d_qr": nc.dram_tensor("d_qr", [2 * M, D], dt.float32,
                                   kind="ExternalOutput").ap(),
            "d_qg": nc.dram_tensor("d_qg", [2 * M, D], dt.bfloat16,
                                   kind="ExternalOutput").ap(),
            "d_qgT": nc.dram_tensor("d_qgT", [D, 2 * M], dt.bfloat16,
                                    kind="ExternalOutput").ap(),
            "d_kb": nc.dram_tensor("d_kb", [128, 2 * NCH, D], dt.bfloat16,
                                   kind="ExternalOutput").ap(),
            "d_kz": nc.dram_tensor("d_kz", [128, 2 * NCH, D], dt.bfloat16,
                                   kind="ExternalOutput").ap(),
            "d_kzT": nc.dram_tensor("d_kzT", [128, 2 * NCH, D], dt.bfloat16,
                                    kind="ExternalOutput").ap(),
            "d_aer": nc.dram_tensor("d_aer", [128, P], dt.bfloat16,
                                    kind="ExternalOutput").ap(),
            "d_ae": nc.dram_tensor("d_ae", [128, P], dt.bfloat16,
                                   kind="ExternalOutput").ap(),
            "d_attT": nc.dram_tensor("d_attT", [128, NCH, 128], dt.bfloat16,
                                     kind="ExternalOutput").ap(),
            "d_ssum": nc.dram_tensor("d_ssum", [128, 1], dt.float32,
                                     kind="ExternalOutput").ap(),
            "d_rsk": nc.dram_tensor("d_rsk", [128, 2 * NCH], dt.float32,
                                    kind="ExternalOutput").ap(),
            "d_mvT": nc.dram_tensor("d_mvT", [128, 2 * NCH, D], dt.bfloat16,
                                    kind="ExternalOutput").ap(),
        }

    def bcast(dram_ap, parts=128):
        return bass.AP(tensor=dram_ap.tensor, offset=dram_ap.offset,
                       ap=[[0, parts]] + list(dram_ap.ap[1:]))

    with tile.TileContext(nc) as tc, ExitStack() as ctx:
        singles = ctx.enter_context(tc.tile_pool(name="singles", bufs=1))
        mvp = ctx.enter_context(tc.tile_pool(name="mvp", bufs=3))
        kp = ctx.enter_context(tc.tile_pool(name="kp", bufs=3))
        qp = ctx.enter_context(tc.tile_pool(name="qp", bufs=3))
        ap_ = ctx.enter_context(tc.tile_pool(name="ap", bufs=3))
        smal = ctx.enter_context(tc.tile_pool(name="smal", bufs=4))
        ps_kg = ctx.enter_context(tc.tile_pool(name="ps_kg", bufs=2, space="PSUM"))
        ps_att = ctx.enter_context(tc.tile_pool(name="ps_att", bufs=1, space="PSUM"))
        ps_sm = ctx.enter_context(tc.tile_pool(name="ps_sm", bufs=1, space="PSUM"))

        # ---- resident constants (k-chain deps first, then q deps) ----
        wqt = singles.tile([D, D], dt.bfloat16)
        nc.sync.dma_start(out=wqt, in_=wqt_in)
        # tT_all [d, pair, m128] via one DRAM transpose (2D out AP: row=free)
        tT_all = singles.tile([D, NPAIR, 2 * M], dt.bfloat16)
        nc.sync.dma_start(out=tT_all.rearrange("d i m -> d (i m)"),
                          in_=t_in, transpose=True)
        wkt = singles.tile([D, D], dt.bfloat16)
        nc.sync.dma_start(out=wkt, in_=wkt_in)
        ktab = singles.tile([128, 4, NCH, 64], dt.bfloat16)
        nc.sync.dma_start(out=ktab, in_=ktab_in)
        q4 = singles.tile([2 * M, NPAIR, 4, 64], dt.bfloat16)
        nc.sync.dma_start(out=q4, in_=q4_in)
        g2bc = singles.tile([128, D], dt.float32)
        nc.sync.dma_start(out=g2bc, in_=bcast(g2_in))
        if has_bln:
            bgbc = singles.tile([128, D], dt.float32)
            nc.sync.dma_start(out=bgbc, in_=bcast(bg_in))
            gbbc = singles.tile([128, D], dt.float32)
            nc.sync.dma_start(out=gbbc, in_=bcast(gb_in))
        if has_bq:
            bqbc = singles.tile([128, D], dt.float32)
            nc.sync.dma_start(out=bqbc, in_=bcast(bq_in))
        if has_bk:
            bkbc = singles.tile([128, D], dt.float32)
            nc.sync.dma_start(out=bkbc, in_=bcast(bk_in))
        epst = singles.tile([128, 1], dt.float32)
        nc.vector.memset(epst, EPS)
        # all pairs' exp-bias, resident (16KB/partition); loaded mid-loop so
        # it does not gate the first pairs' mv loads on the DMA chain
        expb = singles.tile([2 * M, NPAIR, P], dt.bfloat16)

        for i in range(NPAIR):
            # ---- loads ----
            mv_b = mvp.tile([128, 2 * NCH, D + 4], dt.bfloat16, tag="mv_b")
            nc.sync.dma_start(
                out=mv_b,
                in_=mvp_in[:, 2 * i:2 * i + 2].rearrange(
                    "p g c d -> p (g c) d"))
            mvT = mvp.tile([D, 2 * NCH, 128], dt.bfloat16, tag="mvT")
            nc.sync.dma_start(
                out=mvT,
                in_=mvt_in[:, 2 * i:2 * i + 2].rearrange(
                    "d g c p -> d (g c) p"))
            if i == 0:
                nc.sync.dma_start(out=expb, in_=expb_in)

            # ---- k path ----
            if stage < 2:
                out_f = qp.tile([128, D], dt.float32, tag="out_f")
                if stage < 1:
                    nc.vector.memset(out_f, 0.0)
                else:
                    nc.vector.tensor_copy(out_f, mv_b[:, 0, 0:D])
                nc.sync.dma_start(
                    out=out_dram[2 * i:2 * i + 2].rearrange(
                        "g m d -> (g m) d"),
                    in_=out_f)
                continue
            k_b = kp.tile([128, 2 * NCH, D], dt.bfloat16, tag="k_b")
            for g in range(2):
                ps_k = ps_kg.tile([128, NCH, D], dt.float32, tag="psk")
                for c in range(NCH):
                    nc.tensor.matmul(ps_k[:, c, :], mvT[:, NCH * g + c, :],
                                     wkt, start=True, stop=True)
                if has_bk:
                    for c in range(NCH):
                        nc.vector.tensor_add(ps_k[:, c, :], ps_k[:, c, :],
                                             bkbc)
                sl = slice(NCH * g, NCH * (g + 1))
                kt1 = kp.tile([128, NCH, 64], dt.bfloat16, tag="kt1")
                kt2 = kp.tile([128, NCH, 64], dt.bfloat16, tag="kt2")
                kt3 = kp.tile([128, NCH, 64], dt.bfloat16, tag="kt3")
                kt4 = kp.tile([128, NCH, 64], dt.bfloat16, tag="kt4")
                nc.vector.tensor_mul(kt1, ktab[:, 0], ps_k[:, :, 0:64])
                nc.vector.tensor_mul(kt2, ktab[:, 1], ps_k[:, :, 64:128])
                nc.vector.tensor_mul(kt3, ktab[:, 2], ps_k[:, :, 0:64])
                nc.vector.tensor_mul(kt4, ktab[:, 3], ps_k[:, :, 64:128])
                nc.gpsimd.tensor_add(k_b[:, sl, 0:64], kt1, kt2)
                nc.gpsimd.tensor_add(k_b[:, sl, 64:128], kt3, kt4)
            # LN stats: mu, var per (p, chunk)
            ksq = kp.tile([128, 2 * NCH, D], dt.bfloat16, tag="ksq")
            nc.scalar.activation(out=ksq, in_=k_b, func=ACTF.Square)
            s1 = smal.tile([128, 2 * NCH], dt.float32, tag="s1")
            nc.vector.tensor_reduce(out=s1, in_=k_b,
                                    axis=mybir.AxisListType.X, op=ALU.add)
            s2 = smal.tile([128, 2 * NCH], dt.float32, tag="s2")
            nc.vector.tensor_reduce(out=s2, in_=ksq,
                                    axis=mybir.AxisListType.X, op=ALU.add)
            mu = smal.tile([128, 2 * NCH], dt.float32, tag="mu")
            nc.vector.tensor_scalar_mul(mu, s1, 1.0 / D)
            musq = smal.tile([128, 2 * NCH], dt.float32, tag="musq")
            nc.vector.tensor_mul(musq, mu, mu)
            var = smal.tile([128, 2 * NCH], dt.float32, tag="var")
            nc.vector.tensor_scalar_mul(var, s2, 1.0 / D)
            nc.vector.tensor_sub(var, var, musq)
            lnk = smal.tile([128, 2 * NCH], dt.float32, tag="lnk")
            nc.scalar.activation(out=lnk, in_=var, func=ACTF.Ln,
                                 bias=epst, scale=1.0)
            rsk = smal.tile([128, 2 * NCH], dt.float32, tag="rsk")
            nc.scalar.activation(out=rsk, in_=lnk, func=ACTF.Exp, scale=-0.5)
            nmr = smal.tile([128, 2 * NCH], dt.float32, tag="nmr")
            nc.vector.tensor_mul(nmr, mu, rsk)
            nc.vector.tensor_scalar_mul(nmr, nmr, -1.0)
            # kz = k*rstd + nmr: 12 chunks on Scalar (Identity with per-chunk
            # scale/bias), 4 on GpSimd, to shorten the serial chain
            kz = kp.tile([128, 2 * NCH, D], dt.bfloat16, tag="kz")
            for c in range(2 * NCH):
                if c % 4 == 3:
                    nc.gpsimd.tensor_scalar(
                        out=kz[:, c, :], in0=k_b[:, c, :],
                        scalar1=rsk[:, c:c + 1], scalar2=nmr[:, c:c + 1],
                        op0=ALU.mult, op1=ALU.add)
                else:
                    nc.scalar.activation(out=kz[:, c, :], in_=k_b[:, c, :],
                                         func=ACTF.Identity,
                                         scale=rsk[:, c:c + 1],
                                         bias=nmr[:, c:c + 1])
            kzT = kp.tile([128, 2 * NCH, D], dt.bfloat16, tag="kzT")
            nc.sync.dma_start(out=kzT,
                              in_=kz.rearrange("p c d -> p (c d)"),
                              transpose=True)

            # ---- q path ----
            ps_q = ps_sm.tile([128, 128], dt.float32, tag="psq")
            nc.tensor.matmul(ps_q, tT_all[:, i, :], wqt, start=True, stop=True)
            if has_bq:
                nc.vector.tensor_add(ps_q, ps_q, bqbc)
            qt1 = qp.tile([2 * M, 64], dt.float32, tag="qt1")
            qt2 = qp.tile([2 * M, 64], dt.float32, tag="qt2")
            qt3 = qp.tile([2 * M, 64], dt.float32, tag="qt3")
            qt4 = qp.tile([2 * M, 64], dt.float32, tag="qt4")
            nc.vector.tensor_mul(qt1, q4[:, i, 0, :], ps_q[:, 0:64])
            nc.vector.tensor_mul(qt2, q4[:, i, 1, :], ps_q[:, 64:128])
            nc.vector.tensor_mul(qt3, q4[:, i, 2, :], ps_q[:, 0:64])
            nc.vector.tensor_mul(qt4, q4[:, i, 3, :], ps_q[:, 64:128])
            q_r = qp.tile([2 * M, D], dt.float32, tag="q_r")
            nc.vector.tensor_add(q_r[:, 0:64], qt1, qt2)
            nc.vector.tensor_add(q_r[:, 64:128], qt3, qt4)
            bnq = smal.tile([2 * M, 6], dt.float32, tag="bnq")
            nc.vector.bn_stats(out=bnq, in_=q_r)
            mvq = smal.tile([2 * M, 2], dt.float32, tag="mvq")
            nc.vector.bn_aggr(out=mvq, in_=bnq)
            lnq = smal.tile([2 * M, 1], dt.float32, tag="lnq")
            nc.scalar.activation(out=lnq, in_=mvq[:, 1:2], func=ACTF.Ln,
                                 bias=epst[0:2 * M], scale=1.0)
            rsq = smal.tile([2 * M, 1], dt.float32, tag="rsq")
            nc.scalar.activation(out=rsq, in_=lnq, func=ACTF.Exp, scale=-0.5)
            qz = qp.tile([2 * M, D], dt.float32, tag="qz")
            nc.vector.tensor_scalar(out=qz, in0=q_r, scalar1=mvq[:, 0:1],
                                    scalar2=rsq, op0=ALU.subtract,
                                    op1=ALU.mult)
            qg = qp.tile([2 * M, D], dt.bfloat16, tag="qg")
            nc.vector.tensor_mul(qg, qz, g2bc)
            if has_bln:
                nc.vector.tensor_add(qg, qg, bgbc)
                cexp = smal.tile([2 * M, 1], dt.float32, tag="cexp")
                trash = qp.tile([2 * M, D], dt.float32, tag="trash")
                nc.vector.tensor_tensor_reduce(
                    out=trash, in0=qz, in1=gbbc, scale=1.0, scalar=0.0,
                    op0=ALU.mult, op1=ALU.add, accum_out=cexp)
            qgT = qp.tile([D, 2 * M], dt.bfloat16, tag="qgT")
            nc.sync.dma_start(out=qgT, in_=qg, transpose=True)

            # ---- attention ----
            if stage < 3:
                out_f = qp.tile([128, D], dt.float32, tag="out_f")
                nc.vector.tensor_copy(out_f, kzT[:, 0, :])
                nc.sync.dma_start(
                    out=out_dram[2 * i:2 * i + 2].rearrange(
                        "g m d -> (g m) d"),
                    in_=out_f)
                continue
            ps_a = ps_att.tile([128, P], dt.float32, tag="att")
            for gi in range(2):
                lhs = qgT[:, gi * M:(gi + 1) * M]
                for h in range(2):
                    rhs = kzT[:, NCH * gi + 4 * h: NCH * gi + 4 * (h + 1), :]
                    nc.tensor.matmul(
                        ps_a[gi * M:(gi + 1) * M, 512 * h:512 * (h + 1)],
                        lhs, rhs, start=True, stop=True)
            if stage == 21:
                out_f = qp.tile([128, D], dt.float32, tag="out_f")
                nc.vector.tensor_copy(out_f, ps_a[:, 0:D])
                nc.sync.dma_start(
                    out=out_dram[2 * i:2 * i + 2].rearrange(
                        "g m d -> (g m) d"),
                    in_=out_f)
                continue
            aer = ap_.tile([128, P], dt.bfloat16, tag="aer")
            if has_bln:
                nc.scalar.activation(out=aer, in_=ps_a, func=ACTF.Exp,
                                     bias=cexp, scale=1.0)
            else:
                nc.scalar.activation(out=aer, in_=ps_a, func=ACTF.Exp,
                                     scale=1.0)
            if stage == 22:
                out_f = qp.tile([128, D], dt.float32, tag="out_f")
                nc.vector.tensor_copy(out_f, aer[:, 0:D])
                nc.sync.dma_start(
                    out=out_dram[2 * i:2 * i + 2].rearrange(
                        "g m d -> (g m) d"),
                    in_=out_f)
                continue
            ae = ap_.tile([128, P], dt.bfloat16, tag="ae")
            nc.gpsimd.tensor_mul(ae, aer, expb[:, i, :])
            if stage == 23:
                out_f = qp.tile([128, D], dt.float32, tag="out_f")
                nc.vector.tensor_copy(out_f, ae[:, 0:D])
                nc.sync.dma_start(
                    out=out_dram[2 * i:2 * i + 2].rearrange(
                        "g m d -> (g m) d"),
                    in_=out_f)
                continue
            attT = ap_.tile([128, NCH, 128], dt.bfloat16, tag="attT")
            nc.sync.dma_start(out=attT, in_=ae, transpose=True)

            # ---- out = attT.T @ mv ----
            if stage < 4:
                out_f = qp.tile([128, D], dt.float32, tag="out_f")
                nc.vector.tensor_copy(out_f, attT[:, 0, :])
                nc.sync.dma_start(
                    out=out_dram[2 * i:2 * i + 2].rearrange(
                        "g m d -> (g m) d"),
                    in_=out_f)
                continue
            # out cols 0..127 = unnormalized out; col 128 = softmax denom
            ps_o = ps_sm.tile([128, D + 1], dt.float32, tag="out")
            for gi in range(2):
                for c in range(NCH):
                    nc.tensor.matmul(
                        ps_o[gi * M:(gi + 1) * M, :],
                        attT[:, c, gi * M:(gi + 1) * M],
                        mv_b[:, NCH * gi + c, 0:D + 1],
                        start=(c == 0), stop=(c == NCH - 1))
            srec = smal.tile([128, 1], dt.float32, tag="srec")
            nc.vector.reciprocal(srec, ps_o[:, D:D + 1])
            out_f = qp.tile([128, D], dt.float32, tag="out_f")
            nc.vector.tensor_scalar(out=out_f, in0=ps_o[:, 0:D],
                                    scalar1=srec, scalar2=None, op0=ALU.mult)
            nc.sync.dma_start(
                out=out_dram[2 * i:2 * i + 2].rearrange("g m d -> (g m) d"),
                in_=out_f)
            if debug and i == 0:
                nc.sync.dma_start(out=dbg["d_qr"], in_=q_r)
                nc.sync.dma_start(out=dbg["d_qg"], in_=qg)
                nc.sync.dma_start(out=dbg["d_qgT"], in_=qgT)
                nc.sync.dma_start(out=dbg["d_kb"], in_=k_b)
                nc.sync.dma_start(out=dbg["d_kz"], in_=kz)
                nc.sync.dma_start(out=dbg["d_kzT"], in_=kzT)
                nc.sync.dma_start(out=dbg["d_aer"], in_=aer)
                nc.sync.dma_start(out=dbg["d_ae"], in_=ae)
                nc.sync.dma_start(out=dbg["d_attT"], in_=attT)
                nc.sync.dma_start(out=dbg["d_ssum"], in_=srec)
                nc.sync.dma_start(out=dbg["d_rsk"], in_=rsk)
                nc.sync.dma_start(out=dbg["d_mvT"], in_=mvT)

    # Pin one activation table set: strip 'exp'/'ln' from every act-func set
    # except natural_log_exp_and_others (list order/length preserved, so the
    # emitted act_func_set_id still indexes the real act_info.json), making
    # the greedy table-load pass choose the shared set for both Ln and Exp —
    # one ACT_TABLE_LOAD for the whole kernel instead of two per pair.
    import concourse.bacc as bacc_mod
    orig_get = bacc_mod.get_activation_tables

    def pinned_get(arch):
        tabs = orig_get(arch)
        out = {}
        for name, funcs in tabs.items():
            if name != "natural_log_exp_and_others":
                funcs = {f for f in funcs
                         if f not in (mybir.ActivationFunctionType.Exp,
                                      mybir.ActivationFunctionType.Ln)}
            out[name] = funcs
        return out

    bacc_mod.get_activation_tables = pinned_get
    try:
        nc.compile()
    finally:
        bacc_mod.get_activation_tables = orig_get
    return nc


_PROG_CACHE = {}

LAST_RESULT = None


def kernel(t, mv, positions, Wq, bq, Wk, bk, ln_g, ln_b, _trace=False):
    global LAST_RESULT
    from concourse.bass_utils import run_bass_kernel_spmd

    t = np.asarray(t, F32).reshape(BT, M, D)
    mv_a = np.ascontiguousarray(np.asarray(mv, F32).reshape(BT, P, D)).astype(BF16)
    pos = np.asarray(positions).reshape(BT, M).astype(np.int64)
    Wq = np.asarray(Wq, F32)
    Wk = np.asarray(Wk, F32)
    bq = np.asarray(bq, F32)
    bk = np.asarray(bk, F32)
    ln_g = np.asarray(ln_g, F32)
    ln_b = np.asarray(ln_b, F32)

    wqt = np.ascontiguousarray(Wq.T[:, PERM]).astype(BF16)
    wkt = np.ascontiguousarray(Wk.T[:, PERM]).astype(BF16)
    g_p = ln_g[PERM]
    b_p = ln_b[PERM]
    bq_p = bq[PERM].astype(F32)
    bk_p = bk[PERM].astype(F32)
    g2v = (g_p * g_p / SQD).astype(F32)[None, :]
    bgv = (b_p * g_p / SQD).astype(F32)[None, :]
    gbv = (g_p * b_p / SQD).astype(F32)[None, :]

    has_bq = bool(np.any(bq_p))
    has_bk = bool(np.any(bk_p))
    has_bln = bool(np.any(b_p))

    key = (has_bq, has_bk, has_bln)
    if key not in _PROG_CACHE:
        _PROG_CACHE[key] = _build_program(*key)
    nc = _PROG_CACHE[key]

    ktab = _host_k_tables()
    t16 = np.ascontiguousarray(t.reshape(N_CORES, BT_LOC * M, D)).astype(BF16)

    in_maps = []
    for ci in range(N_CORES):
        sl = slice(ci * BT_LOC, (ci + 1) * BT_LOC)
        pos_loc = pos[sl]
        mv_chunked = mv_a[sl].reshape(BT_LOC, NCH, 128, D)
        mvp16 = np.zeros((128, BT_LOC, NCH, D + 4), BF16)
        mvp16[..., 0:D] = mv_chunked.transpose(2, 0, 1, 3)
        mvp16[..., D] = 1.0
        im = {
            "t16": t16[ci],
            "mvp16": mvp16,
            "mvt16": np.ascontiguousarray(mv_chunked.transpose(3, 0, 1, 2)),
            "wqt": wqt, "wkt": wkt,
            "ktab": ktab,
            "q4": _host_q_tables(pos_loc),
            "expb": _host_expbias(pos_loc),
            "g2v": g2v,
        }
        if has_bln:
            im["bgv"] = bgv
            im["gbv"] = gbv
        if has_bq:
            im["bqv"] = bq_p[None, :]
        if has_bk:
            im["bkv"] = bk_p[None, :]
        in_maps.append(im)

    res = run_bass_kernel_spmd(nc, in_maps, core_ids=list(range(N_CORES)),
                               trace=_trace)
    LAST_RESULT = res
    out = np.concatenate([r["out"].reshape(BT_LOC, M, D) for r in res.results])
    return out.reshape(B, T, M, D).astype(F32)
